# revision 17
# baseline (speedup 1.0000x reference)
"""Trainium2 Bass kernel for nn_Actor (gnn_message_passing).

Math (per batch b):
  k_mu = kv[..., :128], v_mu = kv[..., 128:256]
  rel[n,m]  = <k_mu[n], v_mu[m]> / sqrt(128)
  P[n,m,:]  = pos[n] - pos[m];  Pn = P / (||P|| + eps)
  out[n,:]  = 0.01 * tanh( sum_m Pn[n,m,:] * rel[n,m] )

Factored form used here (avoids materializing [N,N,3]):
  W[n,m]   = rel[n,m] / ||P[n,m]||          (diagonal zeroed)
  out[n,d] = 0.01 * tanh( pos[n,d] * s[n] - (W @ pos)[n,d] ),  s[n] = sum_m W[n,m]

On-device pipeline per core (2 batches, data-parallel over B=16 across 8 cores):
  - relT[m,n] via PE matmuls (fp16 operands, fp32 PSUM accum)
  - d2T[m,n] = |p_n - p_m|^2 via a K=18 fp16 split-precision matmul:
      p = a + b with a = fp16(p), b = fp16(p - a); |p|^2 split into 3 fp16
      parts. All cross products are exact in fp32 PSUM, so the pairwise
      distance keeps ~fp32 accuracy even for very close pairs.
  - ScalarE:  norm = sqrt(d2 + 1e-7)  (NaN for the rare negative d2)
  - VectorE:  custom fused op  W = (rel*y0)*max(C1 - norm*y0, 0),
              y0 = bitcast(~norm)*C0  — 1-NR reciprocal (1/sqrt(E) folded in),
              NaN launders to W=0 via the DVE's NaN-suppressing max
  - PE:       P[4,N] += [pos|1]^T @ W^T  (fp16), accumulated over m-tiles
  - epilogue: transpose P to n-major, combine, single tanh, scale, DMA out
"""

import numpy as np

import concourse.bass as bass
import concourse.bacc as bacc
import concourse.mybir as mybir
import concourse.tile as tile
import concourse.dve_ops as dve_ops
from concourse.bass_utils import run_bass_kernel_spmd
from concourse.dve_spec import Spec, Bin, AluOp, Src0, Src1, C0, C1, Zero, maxx, lower
from concourse.dve_uop import DveOpSpec
from concourse.masks import make_identity

F32 = mybir.dt.float32
F16 = mybir.dt.float16

B, N, CKV = 16, 1024, 259
E = 128
NCORES = 8
BPC = B // NCORES          # batches per core
NT = N // 128              # 128-row tiles per batch
KA = 18                    # augmented contraction size for the d2 matmul
ACTION_SCALE = 0.01
D2_BIAS = 1e-7

# Chebyshev-minimax constants for the 1-NR bit-trick reciprocal over
# u = x*bitcast(~x) in [-4.5, -4]; sqrt(1/sqrt(E)) folded in so that
# W = rel * (1/sqrt(E)) / norm comes out of a single fused op.
_C0_BASE = -0.23549792
_C1_BASE = 2.0017324
_SCALE = 1.0 / np.sqrt(E)
C0_FOLD = float(np.float32(_C0_BASE * np.sqrt(_SCALE)))
C1_FOLD = float(np.float32(_C1_BASE * np.sqrt(_SCALE)))


def _register_custom_op():
    name = "RECIP1_MUL_GNN"
    if name in dve_ops._SUB_OPCODE_FOR_NAME:
        return next(op for op in dve_ops.OPS if op.name == name)

    _n = Bin(AluOp.BITWISE_NOT, Src1, Src1)
    _y0 = _n * C0
    _v = C1 - Src1 * _y0
    _vp = maxx(_v, Zero)
    body = (Src0 * _y0) * _vp

    def _ref(in0, in1, s0, s1, imm2):
        in0 = np.asarray(in0, np.float32)
        in1 = np.asarray(in1, np.float32)
        n = (~in1.view(np.int32)).view(np.float32)
        y0 = n * np.float32(s0)
        v = np.float32(s1) - in1 * y0
        v = np.nan_to_num(v, nan=0.0, posinf=np.inf, neginf=-np.inf)
        vp = np.maximum(v, 0)
        return ((in0 * y0) * vp).astype(np.float32)

    spec = Spec(body=body, reference=_ref)
    opcode = dve_ops._CUSTOM_DVE_ROW_BASE + len(dve_ops.OPS)
    shas = {}
    for ver in ("v3", "v4"):
        try:
            uops = lower(spec, ver=ver)
            shas[ver] = DveOpSpec(
                name=name, opcode=opcode, uops=uops, rd1_en=True
            ).sha(ver)
        except Exception:
            pass
    op = dve_ops.DveOp(name, spec, subdim=False, uops_sha=shas)
    dve_ops.OPS.append(op)
    dve_ops.CUSTOM_DVE_SPECS[name] = spec
    dve_ops._SUB_OPCODE_FOR_NAME[name] = opcode
    return op


RECIP1_MUL_GNN = _register_custom_op()



def build_nc(stage=99):
    nc = bacc.Bacc("TRN2", target_bir_lowering=False, debug=False)
    kv_ext = nc.declare_dram_parameter("kv", [BPC, N, CKV], F32, isOutput=False)
    pos_ext = nc.declare_dram_parameter("positions", [BPC, N, 3], F32, isOutput=False)
    out_ext = nc.declare_dram_parameter("out", [BPC, N, 3], F32, isOutput=True)

    with tile.TileContext(nc) as tc:
        with (
            tc.tile_pool(name="const", bufs=1) as constp,
            tc.tile_pool(name="kv16", bufs=2) as kv16p,
            tc.tile_pool(name="kvT", bufs=2) as kvTp,
            tc.tile_pool(name="aug", bufs=2) as augp,
            tc.tile_pool(name="augT", bufs=2) as augTp,
            tc.tile_pool(name="norm", bufs=3) as normp,
            tc.tile_pool(name="wt", bufs=16) as wtp,
            tc.tile_pool(name="epi", bufs=2) as epip,
            tc.tile_pool(name="psrel", bufs=3, space="PSUM") as psrel,
            tc.tile_pool(name="pspro", bufs=1, space="PSUM") as pspro,
            tc.tile_pool(name="psd2", bufs=2, space="PSUM") as psd2,
            tc.tile_pool(name="psP", bufs=1, space="PSUM") as psP,
        ):
            identity16 = constp.tile([128, 128], F16)
            make_identity(nc, identity16[:, :])
            identity32 = constp.tile([128, 128], F32)
            make_identity(nc, identity32[:, :])
            dmask = constp.tile([128, 128], F16)
            nc.gpsimd.memset(dmask[:, :], 1.0)
            nc.gpsimd.affine_select(
                out=dmask[:, :],
                in_=dmask[:, :],
                compare_op=mybir.AluOpType.not_equal,
                fill=0.0,
                base=0,
                pattern=[[-1, 128]],
                channel_multiplier=1,
            )
            bias_tile = constp.tile([128, 1], F32)
            nc.gpsimd.memset(bias_tile[:, :], D2_BIAS)

            # pre-tanh values for both batches; one tanh at the end keeps a
            # single sqrt->tanh ACT-table transition for the whole kernel
            pre_all = constp.tile([128, BPC, NT, 3], F32)

            # PE warm-up primer: ~24 dependency-free matmuls on the identity
            # trip the HAM clock gate (1.2 -> 2.4 GHz) during the DMA-bound
            # prologue so the real matmul stream runs warm from the start
            warm_in = constp.tile([128, 512], F16)
            nc.vector.memset(warm_in[:, :], 0.0)
            warm_ps = psrel.tile([128, 512], F32, tag="rel")
            for i in range(24):
                nc.tensor.matmul(
                    warm_ps[:, :],
                    lhsT=identity16[:, :],
                    rhs=warm_in[:, :],
                    start=(i == 0),
                    stop=(i == 23),
                )
            warm_sink = constp.tile([128, 1], F32)
            nc.vector.tensor_copy(warm_sink[:, :], warm_ps[:, 0:1])

            for b in range(BPC):
                # ---- load kv, casting f32 -> fp16 in the SWDGE DMA ----
                kv16 = kv16p.tile([128, NT, 2 * E], F16, tag="kv16")
                nc.gpsimd.dma_start(
                    out=kv16[:, :, :],
                    in_=kv_ext[b].rearrange("(t p) c -> p t c", p=128)[:, :, 0 : 2 * E],
                )
                # ---- transpose k/v to [e, n] layout via PE (fp16, 1cyc/row) ----
                kmuT = kvTp.tile([128, N], F16, tag="kmuT")
                vmuT = kvTp.tile([128, N], F16, tag="vmuT")
                kT_ps = pspro.tile([128, N], F16, tag="pro")
                for t in range(NT):
                    nc.tensor.transpose(
                        kT_ps[:, 128 * t : 128 * (t + 1)],
                        kv16[:, t, 0:E],
                        identity16[:, :],
                    )
                nc.vector.tensor_copy(kmuT[:, :], kT_ps[:, :])
                vT_ps = pspro.tile([128, N], F16, tag="pro")
                for t in range(NT):
                    nc.tensor.transpose(
                        vT_ps[:, 128 * t : 128 * (t + 1)],
                        kv16[:, t, E : 2 * E],
                        identity16[:, :],
                    )
                nc.scalar.copy(vmuT[:, :], vT_ps[:, :])

                # ---- build augmented position blocks (n-major, fp16) ----
                # moving rows A: [a(3), b(3), a(3), b(3), 1,1,1, pn2 h/m/l]
                # stationary rows Bm: [-2a(3), -2a(3), -2b(3), -2b(3), pm2 h/m/l, 1,1,1]
                posf = augp.tile([128, NT, 3], F32, tag="posf")
                nc.sync.dma_start(
                    out=posf[:, :, :],
                    in_=pos_ext[b].rearrange("(t p) d -> p t d", p=128),
                )
                A16 = augp.tile([128, NT, KA], F16, tag="A16")
                B16 = augp.tile([128, NT, KA], F16, tag="B16")
                sq3 = augp.tile([128, NT, 3], F32, tag="sq3")
                pn2 = augp.tile([128, NT, 1], F32, tag="pn2")
                t1 = augp.tile([128, NT, 1], F32, tag="t1")

                nc.vector.tensor_copy(A16[:, :, 0:3], posf[:, :, :])      # a
                nc.vector.tensor_sub(A16[:, :, 3:6], posf[:, :, :], A16[:, :, 0:3])
                nc.vector.tensor_copy(A16[:, :, 6:9], A16[:, :, 0:3])
                nc.vector.tensor_copy(A16[:, :, 9:12], A16[:, :, 3:6])
                nc.vector.memset(A16[:, :, 12:15], 1.0)
                nc.vector.tensor_mul(sq3[:, :, :], posf[:, :, :], posf[:, :, :])
                nc.vector.tensor_reduce(
                    out=pn2[:, :, :],
                    in_=sq3[:, :, :],
                    op=mybir.AluOpType.add,
                    axis=mybir.AxisListType.X,
                )
                nc.vector.tensor_copy(A16[:, :, 15:16], pn2[:, :, :])     # h
                nc.vector.tensor_sub(t1[:, :, :], pn2[:, :, :], A16[:, :, 15:16])
                nc.vector.tensor_copy(A16[:, :, 16:17], t1[:, :, :])      # m
                nc.vector.tensor_sub(t1[:, :, :], t1[:, :, :], A16[:, :, 16:17])
                nc.vector.tensor_copy(A16[:, :, 17:18], t1[:, :, :])      # l

                nc.vector.tensor_scalar_mul(B16[:, :, 0:3], A16[:, :, 0:3], -2.0)
                nc.vector.tensor_copy(B16[:, :, 3:6], B16[:, :, 0:3])
                nc.vector.tensor_scalar_mul(B16[:, :, 6:9], A16[:, :, 3:6], -2.0)
                nc.vector.tensor_copy(B16[:, :, 9:12], B16[:, :, 6:9])
                nc.vector.tensor_copy(B16[:, :, 12:15], A16[:, :, 15:18])
                nc.vector.memset(B16[:, :, 15:18], 1.0)

                X = augp.tile([128, NT, 4], F16, tag="X")
                nc.vector.tensor_copy(X[:, :, 0:3], A16[:, :, 0:3])
                nc.vector.memset(X[:, :, 3:4], 1.0)

                # ---- transpose aug blocks to [KA, N] via PE ----
                A_ps = pspro.tile([KA, N], F16, tag="pro")
                for t in range(NT):
                    nc.tensor.transpose(
                        A_ps[:, 128 * t : 128 * (t + 1)], A16[:, t, :], identity16[:, :]
                    )
                A16T = augTp.tile([KA, N], F16, tag="A16T")
                nc.vector.tensor_copy(A16T[:, :], A_ps[:, :])

                B_ps = pspro.tile([KA, N], F16, tag="pro")
                for t in range(NT):
                    nc.tensor.transpose(
                        B_ps[:, 128 * t : 128 * (t + 1)], B16[:, t, :], identity16[:, :]
                    )
                B16T = augTp.tile([KA, N], F16, tag="B16T")
                nc.scalar.copy(B16T[:, :], B_ps[:, :])

                # ---- main loop over m-tiles ----
                P_ps = psP.tile([4, N], F32, tag="P")
                wts = []
                for t in range(NT):
                    normt = normp.tile([128, N], F32)
                    wt = wtp.tile([128, N], F16)
                    for h in range(2):
                        cs = slice(512 * h, 512 * (h + 1))
                        d2_ps = psd2.tile([128, 512], F32, tag="d2")
                        nc.tensor.matmul(
                            d2_ps[:, :],
                            lhsT=B16T[:, 128 * t : 128 * (t + 1)],
                            rhs=A16T[:, cs],
                            start=True,
                            stop=True,
                        )
                        if stage >= 4:
                            nc.scalar.activation(
                                normt[:, cs],
                                d2_ps[:, :],
                                mybir.ActivationFunctionType.Sqrt,
                                bias=bias_tile[:, 0:1],
                                scale=1.0,
                            )
                        rel_ps = psrel.tile([128, 512], F32, tag="rel")
                        nc.tensor.matmul(
                            rel_ps[:, :],
                            lhsT=vmuT[:, 128 * t : 128 * (t + 1)],
                            rhs=kmuT[:, cs],
                            start=True,
                            stop=True,
                        )
                        if stage < 5:
                            continue  # STAGEGATE5
                        nc.vector._custom_dve(
                            RECIP1_MUL_GNN,
                            out=wt[:, cs],
                            in0=rel_ps[:, :],
                            in1=normt[:, cs],
                            s0=C0_FOLD,
                            s1=C1_FOLD,
                        )
                    if stage < 5:
                        continue  # STAGEGATE5b
                    # zero the diagonal block (exact diag kill; also keeps the
                    # NaN->0 laundered entries harmless)
                    nc.gpsimd.tensor_mul(
                        wt[:, 128 * t : 128 * (t + 1)],
                        wt[:, 128 * t : 128 * (t + 1)],
                        dmask[:, :],
                    )
                    wts.append(wt)
                if stage >= 6:
                    # dense deferred P-accumulation phase: all W tiles ready
                    for t in range(NT):
                        for h in range(2):
                            cs = slice(512 * h, 512 * (h + 1))
                            nc.tensor.matmul(
                                P_ps[:, cs],
                                lhsT=X[:, t, :],
                                rhs=wts[t][:, cs],
                                start=(t == 0),
                                stop=(t == NT - 1),
                            )

                if stage < 7:
                    nc.vector.tensor_copy(pre_all[:, b, :, :], posf[:, :, :])
                    continue
                # ---- epilogue: P [4,N] -> n-major, combine ----
                Psb = epip.tile([4, N], F32, tag="Psb")
                nc.scalar.copy(Psb[:, :], P_ps[:, :])
                PT_ps = pspro.tile([128, NT * 4], F32, tag="pro")
                for c in range(NT):
                    nc.tensor.transpose(
                        PT_ps[:, 4 * c : 4 * (c + 1)],
                        Psb[:, 128 * c : 128 * (c + 1)],
                        identity32[0:4, 0:4],
                    )
                PT = epip.tile([128, NT, 4], F32, tag="PT")
                nc.vector.tensor_copy(
                    PT[:, :, :], PT_ps[:, :].rearrange("p (t f) -> p t f", f=4)
                )
                tmp = epip.tile([128, NT, 3], F32, tag="tmp")
                a0, a1 = bass.broadcast_tensor_aps(posf[:, :, :], PT[:, :, 3:4])
                nc.gpsimd.tensor_mul(tmp[:, :, :], a0, a1)
                nc.gpsimd.tensor_sub(pre_all[:, b, :, :], tmp[:, :, :], PT[:, :, 0:3])

            # ---- single tanh + scale + store for both batches ----
            act = constp.tile([128, BPC, NT, 3], F32)
            nc.scalar.activation(
                act[:, :, :, :],
                pre_all[:, :, :, :],
                mybir.ActivationFunctionType.Tanh,
            )
            actf = constp.tile([128, BPC, NT, 3], F32)
            nc.gpsimd.tensor_scalar_mul(actf[:, :, :, :], act[:, :, :, :], ACTION_SCALE)
            for b in range(BPC):
                nc.sync.dma_start(
                    out=out_ext[b].rearrange("(t p) d -> p t d", p=128),
                    in_=actf[:, b, :, :],
                )

    nc.compile()
    return nc


_NC_CACHE = {}


def _get_nc():
    if "nc" not in _NC_CACHE:
        _NC_CACHE["nc"] = build_nc()
    return _NC_CACHE["nc"]


def kernel(**inputs):
    kv = np.ascontiguousarray(np.asarray(inputs["kv"], dtype=np.float32))
    pos = np.ascontiguousarray(np.asarray(inputs["positions"], dtype=np.float32))
    assert kv.shape == (B, N, CKV) and pos.shape == (B, N, 3)
    nc = _get_nc()
    in_maps = [
        {
            "kv": kv[i * BPC : (i + 1) * BPC],
            "positions": pos[i * BPC : (i + 1) * BPC],
        }
        for i in range(NCORES)
    ]
    res = run_bass_kernel_spmd(nc, in_maps, core_ids=list(range(NCORES)))
    outs = res.results
    return np.concatenate([outs[i]["out"] for i in range(NCORES)], axis=0)


if __name__ == "__main__":
    rng = np.random.default_rng(0)
    kv = rng.standard_normal((B, N, CKV), dtype=np.float32)
    pos = rng.standard_normal((B, N, 3), dtype=np.float32)
    out = kernel(kv=kv, positions=pos)
    print("out", out.shape, out.dtype, float(np.abs(out).max()))


# revision 20
# speedup vs baseline: 1.1558x; 1.1558x over previous
"""Trainium2 Bass kernel for nn_Actor (gnn_message_passing).

Math (per batch b):
  k_mu = kv[..., :128], v_mu = kv[..., 128:256]
  rel[n,m]  = <k_mu[n], v_mu[m]> / sqrt(128)
  P[n,m,:]  = pos[n] - pos[m];  Pn = P / (||P|| + eps)
  out[n,:]  = 0.01 * tanh( sum_m Pn[n,m,:] * rel[n,m] )

Factored form used here (avoids materializing [N,N,3]):
  W[n,m]   = rel[n,m] / ||P[n,m]||          (diagonal zeroed)
  out[n,d] = 0.01 * tanh( pos[n,d] * s[n] - (W @ pos)[n,d] ),  s[n] = sum_m W[n,m]

On-device pipeline per core (2 batches, data-parallel over B=16 across 8 cores):
  - relT[m,n] via PE matmuls (fp16 operands, fp32 PSUM accum)
  - d2T[m,n] = |p_n - p_m|^2 via a K=18 fp16 split-precision matmul:
      p = a + b with a = fp16(p), b = fp16(p - a); |p|^2 split into 3 fp16
      parts. All cross products are exact in fp32 PSUM, so the pairwise
      distance keeps ~fp32 accuracy even for very close pairs.
  - ScalarE:  norm = sqrt(d2 + 1e-7)  (NaN for the rare negative d2)
  - VectorE:  custom fused op  W = (rel*y0)*max(C1 - norm*y0, 0),
              y0 = bitcast(~norm)*C0  — 1-NR reciprocal (1/sqrt(E) folded in),
              NaN launders to W=0 via the DVE's NaN-suppressing max
  - PE:       P[4,N] += [pos|1]^T @ W^T  (fp16), accumulated over m-tiles
  - epilogue: transpose P to n-major, combine, single tanh, scale, DMA out
"""

import numpy as np

import concourse.bass as bass
import concourse.bacc as bacc
import concourse.mybir as mybir
import concourse.tile as tile
import concourse.dve_ops as dve_ops
from concourse.bass_utils import run_bass_kernel_spmd
from concourse.dve_spec import Spec, Bin, AluOp, Src0, Src1, C0, C1, Zero, maxx, lower
from concourse.dve_uop import DveOpSpec
from concourse.masks import make_identity

F32 = mybir.dt.float32
F16 = mybir.dt.float16

B, N, CKV = 16, 1024, 259
E = 128
NCORES = 8
BPC = B // NCORES          # batches per core
NT = N // 128              # 128-row tiles per batch
KA = 18                    # augmented contraction size for the d2 matmul
ACTION_SCALE = 0.01
D2_BIAS = 1e-7

# Chebyshev-minimax constants for the 1-NR bit-trick reciprocal over
# u = x*bitcast(~x) in [-4.5, -4]; sqrt(1/sqrt(E)) folded in so that
# W = rel * (1/sqrt(E)) / norm comes out of a single fused op.
_C0_BASE = -0.23549792
_C1_BASE = 2.0017324
_SCALE = 1.0 / np.sqrt(E)
C0_FOLD = float(np.float32(_C0_BASE * np.sqrt(_SCALE)))
C1_FOLD = float(np.float32(_C1_BASE * np.sqrt(_SCALE)))


def _register_custom_op():
    name = "RECIP1_MUL_GNN"
    if name in dve_ops._SUB_OPCODE_FOR_NAME:
        return next(op for op in dve_ops.OPS if op.name == name)

    _n = Bin(AluOp.BITWISE_NOT, Src1, Src1)
    _y0 = _n * C0
    _v = C1 - Src1 * _y0
    _vp = maxx(_v, Zero)
    body = (Src0 * _y0) * _vp

    def _ref(in0, in1, s0, s1, imm2):
        in0 = np.asarray(in0, np.float32)
        in1 = np.asarray(in1, np.float32)
        n = (~in1.view(np.int32)).view(np.float32)
        y0 = n * np.float32(s0)
        v = np.float32(s1) - in1 * y0
        v = np.nan_to_num(v, nan=0.0, posinf=np.inf, neginf=-np.inf)
        vp = np.maximum(v, 0)
        return ((in0 * y0) * vp).astype(np.float32)

    spec = Spec(body=body, reference=_ref)
    opcode = dve_ops._CUSTOM_DVE_ROW_BASE + len(dve_ops.OPS)
    shas = {}
    for ver in ("v3", "v4"):
        try:
            uops = lower(spec, ver=ver)
            shas[ver] = DveOpSpec(
                name=name, opcode=opcode, uops=uops, rd1_en=True
            ).sha(ver)
        except Exception:
            pass
    op = dve_ops.DveOp(name, spec, subdim=False, uops_sha=shas)
    dve_ops.OPS.append(op)
    dve_ops.CUSTOM_DVE_SPECS[name] = spec
    dve_ops._SUB_OPCODE_FOR_NAME[name] = opcode
    return op


RECIP1_MUL_GNN = _register_custom_op()



def build_nc(stage=99):
    nc = bacc.Bacc("TRN2", target_bir_lowering=False, debug=False)
    kv_ext = nc.declare_dram_parameter("kv", [BPC, N, CKV], F32, isOutput=False)
    pos_ext = nc.declare_dram_parameter("positions", [BPC, N, 3], F32, isOutput=False)
    out_ext = nc.declare_dram_parameter("out", [BPC, N, 3], F32, isOutput=True)

    with tile.TileContext(nc) as tc:
        with (
            tc.tile_pool(name="const", bufs=1) as constp,
            tc.tile_pool(name="kv16", bufs=2) as kv16p,
            tc.tile_pool(name="kvT", bufs=2) as kvTp,
            tc.tile_pool(name="aug", bufs=2) as augp,
            tc.tile_pool(name="augT", bufs=2) as augTp,
            tc.tile_pool(name="norm", bufs=4) as normp,
            tc.tile_pool(name="wt", bufs=16) as wtp,
            tc.tile_pool(name="epi", bufs=2) as epip,
            tc.tile_pool(name="psrel", bufs=3, space="PSUM") as psrel,
            tc.tile_pool(name="psd2", bufs=2, space="PSUM") as psd2,
            tc.tile_pool(name="pspro", bufs=1, space="PSUM") as pspro,
            tc.tile_pool(name="psP", bufs=1, space="PSUM") as psP,
        ):
            # ---- PE warm-up primer: dependency-free back-to-back matmuls ----
            # (uninitialized operands on purpose: zero waits, so they issue at
            # t=0 and trip the HAM clock gate to 2.4 GHz during the DMA-bound
            # prologue; the product is never consumed mathematically)
            warm_in = constp.tile([128, 512], F16)
            nc.gpsimd.memset(warm_in[:, :], 0.0)
            warm_ps = psrel.tile([128, 512], F32, tag="rel")
            for i in range(20):
                nc.tensor.matmul(
                    warm_ps[:, :],
                    lhsT=warm_in[:, 0:128],
                    rhs=warm_in[:, :],
                    start=(i == 0),
                    stop=(i == 19),
                )
            warm_sink = constp.tile([128, 1], F32)
            nc.vector.tensor_copy(warm_sink[:, :], warm_ps[:, 0:1])

            identity16 = constp.tile([128, 128], F16)
            make_identity(nc, identity16[:, :])
            identity32 = constp.tile([128, 128], F32)
            make_identity(nc, identity32[:, :])
            dmask = constp.tile([128, 128], F16)
            nc.gpsimd.memset(dmask[:, :], 1.0)
            nc.gpsimd.affine_select(
                out=dmask[:, :],
                in_=dmask[:, :],
                compare_op=mybir.AluOpType.not_equal,
                fill=0.0,
                base=0,
                pattern=[[-1, 128]],
                channel_multiplier=1,
            )
            bias_tile = constp.tile([128, 1], F32)
            nc.gpsimd.memset(bias_tile[:, :], D2_BIAS)

            # pre-tanh values for both batches; one tanh at the end keeps a
            # single sqrt->tanh ACT-table transition for the whole kernel
            pre_all = constp.tile([128, BPC, NT, 3], F32)

            kmuT, vmuT, A16T, B16T, X, posf = {}, {}, {}, {}, {}, {}

            # ================= prologue: both batches =================
            for b in range(BPC):
                # ---- load kv, casting f32 -> fp16 in the SWDGE DMA ----
                kv16 = kv16p.tile([128, NT, 2 * E], F16, tag="kv16")
                nc.gpsimd.dma_start(
                    out=kv16[:, :, :],
                    in_=kv_ext[b].rearrange("(t p) c -> p t c", p=128)[:, :, 0 : 2 * E],
                )
                # ---- transpose k/v to [e, n] layout via PE ----
                kmuT[b] = kvTp.tile([128, N], F16, tag="kmuT", name=f"kmuT{b}")
                vmuT[b] = kvTp.tile([128, N], F16, tag="vmuT", name=f"vmuT{b}")
                kT_ps = pspro.tile([128, N], F16, tag="pro")
                for t in range(NT):
                    nc.tensor.transpose(
                        kT_ps[:, 128 * t : 128 * (t + 1)],
                        kv16[:, t, 0:E],
                        identity16[:, :],
                    )
                nc.vector.tensor_copy(kmuT[b][:, :], kT_ps[:, :])
                vT_ps = pspro.tile([128, N], F16, tag="pro")
                for t in range(NT):
                    nc.tensor.transpose(
                        vT_ps[:, 128 * t : 128 * (t + 1)],
                        kv16[:, t, E : 2 * E],
                        identity16[:, :],
                    )
                nc.scalar.copy(vmuT[b][:, :], vT_ps[:, :])

                # ---- build augmented position blocks (n-major, fp16) ----
                # moving rows A: [a(3), b(3), a(3), b(3), 1,1,1, pn2 h/m/l]
                # stationary rows Bm: [-2a(3), -2a(3), -2b(3), -2b(3), pm2 h/m/l, 1,1,1]
                posf[b] = augp.tile([128, NT, 3], F32, tag="posf", name=f"posf{b}")
                nc.sync.dma_start(
                    out=posf[b][:, :, :],
                    in_=pos_ext[b].rearrange("(t p) d -> p t d", p=128),
                )
                pf = posf[b]
                A16 = augp.tile([128, NT, KA], F16, tag="A16")
                B16 = augp.tile([128, NT, KA], F16, tag="B16")
                sq3 = augp.tile([128, NT, 3], F32, tag="sq3")
                pn2 = augp.tile([128, NT, 1], F32, tag="pn2")
                t1 = augp.tile([128, NT, 1], F32, tag="t1")

                nc.vector.tensor_copy(A16[:, :, 0:3], pf[:, :, :])      # a
                nc.vector.tensor_sub(A16[:, :, 3:6], pf[:, :, :], A16[:, :, 0:3])
                nc.vector.tensor_copy(A16[:, :, 6:9], A16[:, :, 0:3])
                nc.vector.tensor_copy(A16[:, :, 9:12], A16[:, :, 3:6])
                nc.vector.memset(A16[:, :, 12:15], 1.0)
                nc.vector.tensor_mul(sq3[:, :, :], pf[:, :, :], pf[:, :, :])
                nc.vector.tensor_reduce(
                    out=pn2[:, :, :],
                    in_=sq3[:, :, :],
                    op=mybir.AluOpType.add,
                    axis=mybir.AxisListType.X,
                )
                nc.vector.tensor_copy(A16[:, :, 15:16], pn2[:, :, :])   # h
                nc.vector.tensor_sub(t1[:, :, :], pn2[:, :, :], A16[:, :, 15:16])
                nc.vector.tensor_copy(A16[:, :, 16:17], t1[:, :, :])    # m
                nc.vector.tensor_sub(t1[:, :, :], t1[:, :, :], A16[:, :, 16:17])
                nc.vector.tensor_copy(A16[:, :, 17:18], t1[:, :, :])    # l

                nc.vector.tensor_scalar_mul(B16[:, :, 0:3], A16[:, :, 0:3], -2.0)
                nc.vector.tensor_copy(B16[:, :, 3:6], B16[:, :, 0:3])
                nc.vector.tensor_scalar_mul(B16[:, :, 6:9], A16[:, :, 3:6], -2.0)
                nc.vector.tensor_copy(B16[:, :, 9:12], B16[:, :, 6:9])
                nc.vector.tensor_copy(B16[:, :, 12:15], A16[:, :, 15:18])
                nc.vector.memset(B16[:, :, 15:18], 1.0)

                X[b] = augp.tile([128, NT, 4], F16, tag="X", name=f"X{b}")
                nc.vector.tensor_copy(X[b][:, :, 0:3], A16[:, :, 0:3])
                nc.vector.memset(X[b][:, :, 3:4], 1.0)

                # ---- transpose aug blocks to [KA, N] via PE ----
                A_ps = pspro.tile([KA, N], F16, tag="pro")
                for t in range(NT):
                    nc.tensor.transpose(
                        A_ps[:, 128 * t : 128 * (t + 1)], A16[:, t, :], identity16[:, :]
                    )
                A16T[b] = augTp.tile([KA, N], F16, tag="A16T", name=f"A16T{b}")
                nc.vector.tensor_copy(A16T[b][:, :], A_ps[:, :])

                B_ps = pspro.tile([KA, N], F16, tag="pro")
                for t in range(NT):
                    nc.tensor.transpose(
                        B_ps[:, 128 * t : 128 * (t + 1)], B16[:, t, :], identity16[:, :]
                    )
                B16T[b] = augTp.tile([KA, N], F16, tag="B16T", name=f"B16T{b}")
                nc.scalar.copy(B16T[b][:, :], B_ps[:, :])

            # ============ main loop: interleave both batches ============
            wts = {b: [] for b in range(BPC)}
            for t in range(NT):
                for b in range(BPC):
                    normt = normp.tile([128, N], F32)
                    wt = wtp.tile([128, N], F16)
                    for h in range(2):
                        cs = slice(512 * h, 512 * (h + 1))
                        d2_ps = psd2.tile([128, 512], F32, tag="d2")
                        nc.tensor.matmul(
                            d2_ps[:, :],
                            lhsT=B16T[b][:, 128 * t : 128 * (t + 1)],
                            rhs=A16T[b][:, cs],
                            start=True,
                            stop=True,
                        )
                        nc.scalar.activation(
                            normt[:, cs],
                            d2_ps[:, :],
                            mybir.ActivationFunctionType.Sqrt,
                            bias=bias_tile[:, 0:1],
                            scale=1.0,
                        )
                        rel_ps = psrel.tile([128, 512], F32, tag="rel")
                        nc.tensor.matmul(
                            rel_ps[:, :],
                            lhsT=vmuT[b][:, 128 * t : 128 * (t + 1)],
                            rhs=kmuT[b][:, cs],
                            start=True,
                            stop=True,
                        )
                        nc.vector._custom_dve(
                            RECIP1_MUL_GNN,
                            out=wt[:, cs],
                            in0=rel_ps[:, :],
                            in1=normt[:, cs],
                            s0=C0_FOLD,
                            s1=C1_FOLD,
                        )
                    # zero the diagonal block (exact diag kill; also keeps the
                    # NaN->0 laundered entries harmless)
                    nc.gpsimd.tensor_mul(
                        wt[:, 128 * t : 128 * (t + 1)],
                        wt[:, 128 * t : 128 * (t + 1)],
                        dmask[:, :],
                    )
                    wts[b].append(wt)

            # ============ deferred P accumulation + epilogues ============
            for b in range(BPC):
                P_ps = psP.tile([4, N], F32, tag="P")
                for t in range(NT):
                    for h in range(2):
                        cs = slice(512 * h, 512 * (h + 1))
                        nc.tensor.matmul(
                            P_ps[:, cs],
                            lhsT=X[b][:, t, :],
                            rhs=wts[b][t][:, cs],
                            start=(t == 0),
                            stop=(t == NT - 1),
                        )
                Psb = epip.tile([4, N], F32, tag="Psb")
                nc.scalar.copy(Psb[:, :], P_ps[:, :])
                PT_ps = pspro.tile([128, NT * 4], F32, tag="pro")
                for c in range(NT):
                    nc.tensor.transpose(
                        PT_ps[:, 4 * c : 4 * (c + 1)],
                        Psb[:, 128 * c : 128 * (c + 1)],
                        identity32[0:4, 0:4],
                    )
                PT = epip.tile([128, NT, 4], F32, tag="PT")
                nc.vector.tensor_copy(
                    PT[:, :, :], PT_ps[:, :].rearrange("p (t f) -> p t f", f=4)
                )
                tmp = epip.tile([128, NT, 3], F32, tag="tmp")
                a0, a1 = bass.broadcast_tensor_aps(posf[b][:, :, :], PT[:, :, 3:4])
                nc.gpsimd.tensor_mul(tmp[:, :, :], a0, a1)
                nc.gpsimd.tensor_sub(pre_all[:, b, :, :], tmp[:, :, :], PT[:, :, 0:3])

            # ---- single tanh + scale + store for both batches ----
            act = constp.tile([128, BPC, NT, 3], F32)
            nc.scalar.activation(
                act[:, :, :, :],
                pre_all[:, :, :, :],
                mybir.ActivationFunctionType.Tanh,
            )
            actf = constp.tile([128, BPC, NT, 3], F32)
            nc.gpsimd.tensor_scalar_mul(actf[:, :, :, :], act[:, :, :, :], ACTION_SCALE)
            for b in range(BPC):
                nc.sync.dma_start(
                    out=out_ext[b].rearrange("(t p) d -> p t d", p=128),
                    in_=actf[:, b, :, :],
                )

    nc.compile()
    return nc


_NC_CACHE = {}


def _get_nc():
    if "nc" not in _NC_CACHE:
        _NC_CACHE["nc"] = build_nc()
    return _NC_CACHE["nc"]


def kernel(**inputs):
    kv = np.ascontiguousarray(np.asarray(inputs["kv"], dtype=np.float32))
    pos = np.ascontiguousarray(np.asarray(inputs["positions"], dtype=np.float32))
    assert kv.shape == (B, N, CKV) and pos.shape == (B, N, 3)
    nc = _get_nc()
    in_maps = [
        {
            "kv": kv[i * BPC : (i + 1) * BPC],
            "positions": pos[i * BPC : (i + 1) * BPC],
        }
        for i in range(NCORES)
    ]
    res = run_bass_kernel_spmd(nc, in_maps, core_ids=list(range(NCORES)))
    outs = res.results
    return np.concatenate([outs[i]["out"] for i in range(NCORES)], axis=0)


if __name__ == "__main__":
    rng = np.random.default_rng(0)
    kv = rng.standard_normal((B, N, CKV), dtype=np.float32)
    pos = rng.standard_normal((B, N, 3), dtype=np.float32)
    out = kernel(kv=kv, positions=pos)
    print("out", out.shape, out.dtype, float(np.abs(out).max()))


# revision 21
# speedup vs baseline: 1.1657x; 1.0085x over previous
"""Trainium2 Bass kernel for nn_Actor (gnn_message_passing).

Math (per batch b):
  k_mu = kv[..., :128], v_mu = kv[..., 128:256]
  rel[n,m]  = <k_mu[n], v_mu[m]> / sqrt(128)
  P[n,m,:]  = pos[n] - pos[m];  Pn = P / (||P|| + eps)
  out[n,:]  = 0.01 * tanh( sum_m Pn[n,m,:] * rel[n,m] )

Factored form used here (avoids materializing [N,N,3]):
  W[n,m]   = rel[n,m] / ||P[n,m]||          (diagonal zeroed)
  out[n,d] = 0.01 * tanh( pos[n,d] * s[n] - (W @ pos)[n,d] ),  s[n] = sum_m W[n,m]

On-device pipeline per core (2 batches, data-parallel over B=16 across 8 cores):
  - relT[m,n] via PE matmuls (fp16 operands, fp32 PSUM accum)
  - d2T[m,n] = |p_n - p_m|^2 via a K=18 fp16 split-precision matmul:
      p = a + b with a = fp16(p), b = fp16(p - a); |p|^2 split into 3 fp16
      parts. All cross products are exact in fp32 PSUM, so the pairwise
      distance keeps ~fp32 accuracy even for very close pairs.
  - ScalarE:  norm = sqrt(d2 + 1e-7)  (NaN for the rare negative d2)
  - VectorE:  custom fused op  W = (rel*y0)*max(C1 - norm*y0, 0),
              y0 = bitcast(~norm)*C0  — 1-NR reciprocal (1/sqrt(E) folded in),
              NaN launders to W=0 via the DVE's NaN-suppressing max
  - PE:       P[4,N] += [pos|1]^T @ W^T  (fp16), accumulated over m-tiles
  - epilogue: transpose P to n-major, combine, single tanh, scale, DMA out
"""

import numpy as np

import concourse.bass as bass
import concourse.bacc as bacc
import concourse.mybir as mybir
import concourse.tile as tile
import concourse.dve_ops as dve_ops
from concourse.bass_utils import run_bass_kernel_spmd
from concourse.dve_spec import Spec, Bin, AluOp, Src0, Src1, C0, C1, Zero, maxx, lower
from concourse.dve_uop import DveOpSpec
from concourse.masks import make_identity

F32 = mybir.dt.float32
F16 = mybir.dt.float16

B, N, CKV = 16, 1024, 259
E = 128
NCORES = 8
BPC = B // NCORES          # batches per core
NT = N // 128              # 128-row tiles per batch
KA = 18                    # augmented contraction size for the d2 matmul
ACTION_SCALE = 0.01
D2_BIAS = 1e-7

# Chebyshev-minimax constants for the 1-NR bit-trick reciprocal over
# u = x*bitcast(~x) in [-4.5, -4]; sqrt(1/sqrt(E)) folded in so that
# W = rel * (1/sqrt(E)) / norm comes out of a single fused op.
_C0_BASE = -0.23549792
_C1_BASE = 2.0017324
_SCALE = 1.0 / np.sqrt(E)
C0_FOLD = float(np.float32(_C0_BASE * np.sqrt(_SCALE)))
C1_FOLD = float(np.float32(_C1_BASE * np.sqrt(_SCALE)))


def _register_custom_op():
    name = "RECIP1_MUL_GNN"
    if name in dve_ops._SUB_OPCODE_FOR_NAME:
        return next(op for op in dve_ops.OPS if op.name == name)

    _n = Bin(AluOp.BITWISE_NOT, Src1, Src1)
    _y0 = _n * C0
    _v = C1 - Src1 * _y0
    _vp = maxx(_v, Zero)
    body = (Src0 * _y0) * _vp

    def _ref(in0, in1, s0, s1, imm2):
        in0 = np.asarray(in0, np.float32)
        in1 = np.asarray(in1, np.float32)
        n = (~in1.view(np.int32)).view(np.float32)
        y0 = n * np.float32(s0)
        v = np.float32(s1) - in1 * y0
        v = np.nan_to_num(v, nan=0.0, posinf=np.inf, neginf=-np.inf)
        vp = np.maximum(v, 0)
        return ((in0 * y0) * vp).astype(np.float32)

    spec = Spec(body=body, reference=_ref)
    opcode = dve_ops._CUSTOM_DVE_ROW_BASE + len(dve_ops.OPS)
    shas = {}
    for ver in ("v3", "v4"):
        try:
            uops = lower(spec, ver=ver)
            shas[ver] = DveOpSpec(
                name=name, opcode=opcode, uops=uops, rd1_en=True
            ).sha(ver)
        except Exception:
            pass
    op = dve_ops.DveOp(name, spec, subdim=False, uops_sha=shas)
    dve_ops.OPS.append(op)
    dve_ops.CUSTOM_DVE_SPECS[name] = spec
    dve_ops._SUB_OPCODE_FOR_NAME[name] = opcode
    return op


RECIP1_MUL_GNN = _register_custom_op()



def build_nc(stage=99):
    nc = bacc.Bacc("TRN2", target_bir_lowering=False, debug=False)
    kv_ext = nc.declare_dram_parameter("kv", [BPC, N, CKV], F32, isOutput=False)
    pos_ext = nc.declare_dram_parameter("positions", [BPC, N, 3], F32, isOutput=False)
    out_ext = nc.declare_dram_parameter("out", [BPC, N, 3], F32, isOutput=True)

    with tile.TileContext(nc) as tc:
        with (
            tc.tile_pool(name="const", bufs=1) as constp,
            tc.tile_pool(name="kv16", bufs=2) as kv16p,
            tc.tile_pool(name="kvT", bufs=2) as kvTp,
            tc.tile_pool(name="aug", bufs=2) as augp,
            tc.tile_pool(name="augT", bufs=2) as augTp,
            tc.tile_pool(name="norm", bufs=4) as normp,
            tc.tile_pool(name="wt", bufs=16) as wtp,
            tc.tile_pool(name="epi", bufs=2) as epip,
            tc.tile_pool(name="psrel", bufs=3, space="PSUM") as psrel,
            tc.tile_pool(name="psd2", bufs=2, space="PSUM") as psd2,
            tc.tile_pool(name="pspro", bufs=1, space="PSUM") as pspro,
            tc.tile_pool(name="psP", bufs=1, space="PSUM") as psP,
        ):
            # ---- PE warm-up primer: dependency-free back-to-back matmuls ----
            # (uninitialized operands on purpose: zero waits, so they issue at
            # t=0 and trip the HAM clock gate to 2.4 GHz during the DMA-bound
            # prologue; the product is never consumed mathematically)
            warm_in = constp.tile([128, 512], F16)
            nc.gpsimd.memset(warm_in[:, :], 0.0)
            warm_ps = psrel.tile([128, 512], F32, tag="rel")
            for i in range(20):
                nc.tensor.matmul(
                    warm_ps[:, :],
                    lhsT=warm_in[:, 0:128],
                    rhs=warm_in[:, :],
                    start=(i == 0),
                    stop=(i == 19),
                )
            warm_sink = constp.tile([128, 1], F32)
            nc.vector.tensor_copy(warm_sink[:, :], warm_ps[:, 0:1])

            identity16 = constp.tile([128, 128], F16)
            make_identity(nc, identity16[:, :])
            identity32 = constp.tile([128, 128], F32)
            make_identity(nc, identity32[:, :])
            dmask = constp.tile([128, 128], F16)
            nc.gpsimd.memset(dmask[:, :], 1.0)
            nc.gpsimd.affine_select(
                out=dmask[:, :],
                in_=dmask[:, :],
                compare_op=mybir.AluOpType.not_equal,
                fill=0.0,
                base=0,
                pattern=[[-1, 128]],
                channel_multiplier=1,
            )
            bias_tile = constp.tile([128, 1], F32)
            nc.gpsimd.memset(bias_tile[:, :], D2_BIAS)

            # pre-tanh values for both batches; one tanh at the end keeps a
            # single sqrt->tanh ACT-table transition for the whole kernel
            pre_all = constp.tile([128, BPC, NT, 3], F32)

            kmuT, vmuT, A16T, B16T, X, posf = {}, {}, {}, {}, {}, {}

            # ================= prologue: both batches =================
            for b in range(BPC):
                # ---- load kv, casting f32 -> fp16 in the SWDGE DMA ----
                kv16 = kv16p.tile([128, NT, 2 * E], F16, tag="kv16")
                nc.gpsimd.dma_start(
                    out=kv16[:, :, :],
                    in_=kv_ext[b].rearrange("(t p) c -> p t c", p=128)[:, :, 0 : 2 * E],
                )
                # ---- transpose k/v to [e, n] layout via PE ----
                kmuT[b] = kvTp.tile([128, N], F16, tag="kmuT", name=f"kmuT{b}")
                vmuT[b] = kvTp.tile([128, N], F16, tag="vmuT", name=f"vmuT{b}")
                kT_ps = pspro.tile([128, N], F16, tag="pro")
                for t in range(NT):
                    nc.tensor.transpose(
                        kT_ps[:, 128 * t : 128 * (t + 1)],
                        kv16[:, t, 0:E],
                        identity16[:, :],
                    )
                nc.vector.tensor_copy(kmuT[b][:, :], kT_ps[:, :])
                vT_ps = pspro.tile([128, N], F16, tag="pro")
                for t in range(NT):
                    nc.tensor.transpose(
                        vT_ps[:, 128 * t : 128 * (t + 1)],
                        kv16[:, t, E : 2 * E],
                        identity16[:, :],
                    )
                nc.scalar.copy(vmuT[b][:, :], vT_ps[:, :])

                # ---- build augmented position blocks (n-major, fp16) ----
                # moving rows A: [a(3), b(3), a(3), b(3), 1,1,1, pn2 h/m/l]
                # stationary rows Bm: [-2a(3), -2a(3), -2b(3), -2b(3), pm2 h/m/l, 1,1,1]
                posf[b] = augp.tile([128, NT, 3], F32, tag="posf", name=f"posf{b}")
                nc.sync.dma_start(
                    out=posf[b][:, :, :],
                    in_=pos_ext[b].rearrange("(t p) d -> p t d", p=128),
                )
                pf = posf[b]
                A16 = augp.tile([128, NT, KA], F16, tag="A16")
                B16 = augp.tile([128, NT, KA], F16, tag="B16")
                sq3 = augp.tile([128, NT, 3], F32, tag="sq3")
                pn2 = augp.tile([128, NT, 1], F32, tag="pn2")
                t1 = augp.tile([128, NT, 1], F32, tag="t1")

                nc.vector.tensor_copy(A16[:, :, 0:3], pf[:, :, :])      # a
                nc.vector.tensor_sub(A16[:, :, 3:6], pf[:, :, :], A16[:, :, 0:3])
                nc.vector.tensor_copy(A16[:, :, 6:9], A16[:, :, 0:3])
                nc.vector.tensor_copy(A16[:, :, 9:12], A16[:, :, 3:6])
                nc.vector.memset(A16[:, :, 12:15], 1.0)
                nc.vector.tensor_mul(sq3[:, :, :], pf[:, :, :], pf[:, :, :])
                nc.vector.tensor_reduce(
                    out=pn2[:, :, :],
                    in_=sq3[:, :, :],
                    op=mybir.AluOpType.add,
                    axis=mybir.AxisListType.X,
                )
                nc.vector.tensor_copy(A16[:, :, 15:16], pn2[:, :, :])   # h
                nc.vector.tensor_sub(t1[:, :, :], pn2[:, :, :], A16[:, :, 15:16])
                nc.vector.tensor_copy(A16[:, :, 16:17], t1[:, :, :])    # m
                nc.vector.tensor_sub(t1[:, :, :], t1[:, :, :], A16[:, :, 16:17])
                nc.vector.tensor_copy(A16[:, :, 17:18], t1[:, :, :])    # l

                nc.vector.tensor_scalar_mul(B16[:, :, 0:3], A16[:, :, 0:3], -2.0)
                nc.vector.tensor_copy(B16[:, :, 3:6], B16[:, :, 0:3])
                nc.vector.tensor_scalar_mul(B16[:, :, 6:9], A16[:, :, 3:6], -2.0)
                nc.vector.tensor_copy(B16[:, :, 9:12], B16[:, :, 6:9])
                nc.vector.tensor_copy(B16[:, :, 12:15], A16[:, :, 15:18])
                nc.vector.memset(B16[:, :, 15:18], 1.0)

                X[b] = augp.tile([128, NT, 4], F16, tag="X", name=f"X{b}")
                nc.vector.tensor_copy(X[b][:, :, 0:3], A16[:, :, 0:3])
                nc.vector.memset(X[b][:, :, 3:4], 1.0)

                # ---- transpose aug blocks to [KA, N] via PE ----
                A_ps = pspro.tile([KA, N], F16, tag="pro")
                for t in range(NT):
                    nc.tensor.transpose(
                        A_ps[:, 128 * t : 128 * (t + 1)], A16[:, t, :], identity16[:, :]
                    )
                A16T[b] = augTp.tile([KA, N], F16, tag="A16T", name=f"A16T{b}")
                nc.vector.tensor_copy(A16T[b][:, :], A_ps[:, :])

                B_ps = pspro.tile([KA, N], F16, tag="pro")
                for t in range(NT):
                    nc.tensor.transpose(
                        B_ps[:, 128 * t : 128 * (t + 1)], B16[:, t, :], identity16[:, :]
                    )
                B16T[b] = augTp.tile([KA, N], F16, tag="B16T", name=f"B16T{b}")
                nc.scalar.copy(B16T[b][:, :], B_ps[:, :])

            # ============ main loop: interleave both batches ============
            # b0 leads so the loop never stalls on b1's prologue; b1 trails
            # and b0's deferred P-matmuls fill the PE while b1 finishes
            pair_order = [(0, 0), (0, 1), (0, 2)]
            rest0 = [(0, t) for t in range(3, NT)]
            rest1 = [(1, t) for t in range(NT)]
            while rest0 or rest1:
                if rest1:
                    pair_order.append(rest1.pop(0))
                if rest0:
                    pair_order.append(rest0.pop(0))
            wts = {b: [] for b in range(BPC)}
            for b, t in pair_order:
                if True:
                    normt = normp.tile([128, N], F32)
                    wt = wtp.tile([128, N], F16)
                    for h in range(2):
                        cs = slice(512 * h, 512 * (h + 1))
                        d2_ps = psd2.tile([128, 512], F32, tag="d2")
                        nc.tensor.matmul(
                            d2_ps[:, :],
                            lhsT=B16T[b][:, 128 * t : 128 * (t + 1)],
                            rhs=A16T[b][:, cs],
                            start=True,
                            stop=True,
                        )
                        nc.scalar.activation(
                            normt[:, cs],
                            d2_ps[:, :],
                            mybir.ActivationFunctionType.Sqrt,
                            bias=bias_tile[:, 0:1],
                            scale=1.0,
                        )
                        rel_ps = psrel.tile([128, 512], F32, tag="rel")
                        nc.tensor.matmul(
                            rel_ps[:, :],
                            lhsT=vmuT[b][:, 128 * t : 128 * (t + 1)],
                            rhs=kmuT[b][:, cs],
                            start=True,
                            stop=True,
                        )
                        nc.vector._custom_dve(
                            RECIP1_MUL_GNN,
                            out=wt[:, cs],
                            in0=rel_ps[:, :],
                            in1=normt[:, cs],
                            s0=C0_FOLD,
                            s1=C1_FOLD,
                        )
                    # zero the diagonal block (exact diag kill; also keeps the
                    # NaN->0 laundered entries harmless)
                    nc.gpsimd.tensor_mul(
                        wt[:, 128 * t : 128 * (t + 1)],
                        wt[:, 128 * t : 128 * (t + 1)],
                        dmask[:, :],
                    )
                    wts[b].append(wt)

            # ============ deferred P accumulation + epilogues ============
            for b in range(BPC):
                P_ps = psP.tile([4, N], F32, tag="P", name=f"P{b}")
                for t in range(NT):
                    for h in range(2):
                        cs = slice(512 * h, 512 * (h + 1))
                        nc.tensor.matmul(
                            P_ps[:, cs],
                            lhsT=X[b][:, t, :],
                            rhs=wts[b][t][:, cs],
                            start=(t == 0),
                            stop=(t == NT - 1),
                        )
                Psb = epip.tile([4, N], F32, tag="Psb")
                nc.scalar.copy(Psb[:, :], P_ps[:, :])
                PT_ps = pspro.tile([128, NT * 4], F32, tag="pro")
                for c in range(NT):
                    nc.tensor.transpose(
                        PT_ps[:, 4 * c : 4 * (c + 1)],
                        Psb[:, 128 * c : 128 * (c + 1)],
                        identity32[0:4, 0:4],
                    )
                PT = epip.tile([128, NT, 4], F32, tag="PT")
                nc.vector.tensor_copy(
                    PT[:, :, :], PT_ps[:, :].rearrange("p (t f) -> p t f", f=4)
                )
                tmp = epip.tile([128, NT, 3], F32, tag="tmp")
                a0, a1 = bass.broadcast_tensor_aps(posf[b][:, :, :], PT[:, :, 3:4])
                nc.gpsimd.tensor_mul(tmp[:, :, :], a0, a1)
                nc.gpsimd.tensor_sub(pre_all[:, b, :, :], tmp[:, :, :], PT[:, :, 0:3])

            # ---- single tanh + scale + store for both batches ----
            act = constp.tile([128, BPC, NT, 3], F32)
            nc.scalar.activation(
                act[:, :, :, :],
                pre_all[:, :, :, :],
                mybir.ActivationFunctionType.Tanh,
            )
            actf = constp.tile([128, BPC, NT, 3], F32)
            nc.gpsimd.tensor_scalar_mul(actf[:, :, :, :], act[:, :, :, :], ACTION_SCALE)
            for b in range(BPC):
                nc.sync.dma_start(
                    out=out_ext[b].rearrange("(t p) d -> p t d", p=128),
                    in_=actf[:, b, :, :],
                )

    nc.compile()
    return nc


_NC_CACHE = {}


def _get_nc():
    if "nc" not in _NC_CACHE:
        _NC_CACHE["nc"] = build_nc()
    return _NC_CACHE["nc"]


def kernel(**inputs):
    kv = np.ascontiguousarray(np.asarray(inputs["kv"], dtype=np.float32))
    pos = np.ascontiguousarray(np.asarray(inputs["positions"], dtype=np.float32))
    assert kv.shape == (B, N, CKV) and pos.shape == (B, N, 3)
    nc = _get_nc()
    in_maps = [
        {
            "kv": kv[i * BPC : (i + 1) * BPC],
            "positions": pos[i * BPC : (i + 1) * BPC],
        }
        for i in range(NCORES)
    ]
    res = run_bass_kernel_spmd(nc, in_maps, core_ids=list(range(NCORES)))
    outs = res.results
    return np.concatenate([outs[i]["out"] for i in range(NCORES)], axis=0)


if __name__ == "__main__":
    rng = np.random.default_rng(0)
    kv = rng.standard_normal((B, N, CKV), dtype=np.float32)
    pos = rng.standard_normal((B, N, 3), dtype=np.float32)
    out = kernel(kv=kv, positions=pos)
    print("out", out.shape, out.dtype, float(np.abs(out).max()))


# revision 23
# speedup vs baseline: 1.3006x; 1.1158x over previous
"""Trainium2 Bass kernel for nn_Actor (gnn_message_passing).

Math (per batch b):
  k_mu = kv[..., :128], v_mu = kv[..., 128:256]
  rel[n,m]  = <k_mu[n], v_mu[m]> / sqrt(128)
  P[n,m,:]  = pos[n] - pos[m];  Pn = P / (||P|| + eps)
  out[n,:]  = 0.01 * tanh( sum_m Pn[n,m,:] * rel[n,m] )

Factored form used here (avoids materializing [N,N,3]):
  W[n,m]   = rel[n,m] / ||P[n,m]||          (diagonal zeroed)
  out[n,d] = 0.01 * tanh( pos[n,d] * s[n] - (W @ pos)[n,d] ),  s[n] = sum_m W[n,m]

On-device pipeline per core (2 batches, data-parallel over B=16 across 8 cores):
  - relT[m,n] via PE matmuls (fp16 operands, fp32 PSUM accum)
  - d2T[m,n] = |p_n - p_m|^2 via a K=18 fp16 split-precision matmul:
      p = a + b with a = fp16(p), b = fp16(p - a); |p|^2 split into 3 fp16
      parts. All cross products are exact in fp32 PSUM, so the pairwise
      distance keeps ~fp32 accuracy even for very close pairs.
  - ScalarE:  norm = sqrt(d2 + 1e-7)  (NaN for the rare negative d2)
  - VectorE:  custom fused op  W = (rel*y0)*max(C1 - norm*y0, 0),
              y0 = bitcast(~norm)*C0  — 1-NR reciprocal (1/sqrt(E) folded in),
              NaN launders to W=0 via the DVE's NaN-suppressing max
  - PE:       P[4,N] += [pos|1]^T @ W^T  (fp16), accumulated over m-tiles
  - epilogue: transpose P to n-major, combine, single tanh, scale, DMA out
"""

import numpy as np

import concourse.bass as bass
import concourse.bacc as bacc
import concourse.mybir as mybir
import concourse.tile as tile
import concourse.dve_ops as dve_ops
from concourse.bass_utils import run_bass_kernel_spmd
from concourse.dve_spec import Spec, Bin, AluOp, Src0, Src1, C0, C1, Zero, maxx, lower
from concourse.dve_uop import DveOpSpec
from concourse.masks import make_identity

F32 = mybir.dt.float32
F16 = mybir.dt.float16

B, N, CKV = 16, 1024, 259
E = 128
NCORES = 8
BPC = B // NCORES          # batches per core
NT = N // 128              # 128-row tiles per batch
KA = 18                    # augmented contraction size for the d2 matmul
ACTION_SCALE = 0.01
D2_BIAS = 1e-7

# Chebyshev-minimax constants for the 1-NR bit-trick reciprocal over
# u = x*bitcast(~x) in [-4.5, -4]; sqrt(1/sqrt(E)) folded in so that
# W = rel * (1/sqrt(E)) / norm comes out of a single fused op.
_C0_BASE = -0.23549792
_C1_BASE = 2.0017324
_SCALE = 1.0 / np.sqrt(E)
C0_FOLD = float(np.float32(_C0_BASE * np.sqrt(_SCALE)))
C1_FOLD = float(np.float32(_C1_BASE * np.sqrt(_SCALE)))


def _register_custom_op():
    name = "RECIP1_MUL_GNN"
    if name in dve_ops._SUB_OPCODE_FOR_NAME:
        return next(op for op in dve_ops.OPS if op.name == name)

    _n = Bin(AluOp.BITWISE_NOT, Src1, Src1)
    _y0 = _n * C0
    _v = C1 - Src1 * _y0
    _vp = maxx(_v, Zero)
    body = (Src0 * _y0) * _vp

    def _ref(in0, in1, s0, s1, imm2):
        in0 = np.asarray(in0, np.float32)
        in1 = np.asarray(in1, np.float32)
        n = (~in1.view(np.int32)).view(np.float32)
        y0 = n * np.float32(s0)
        v = np.float32(s1) - in1 * y0
        v = np.nan_to_num(v, nan=0.0, posinf=np.inf, neginf=-np.inf)
        vp = np.maximum(v, 0)
        return ((in0 * y0) * vp).astype(np.float32)

    spec = Spec(body=body, reference=_ref)
    opcode = dve_ops._CUSTOM_DVE_ROW_BASE + len(dve_ops.OPS)
    shas = {}
    for ver in ("v3", "v4"):
        try:
            uops = lower(spec, ver=ver)
            shas[ver] = DveOpSpec(
                name=name, opcode=opcode, uops=uops, rd1_en=True
            ).sha(ver)
        except Exception:
            pass
    op = dve_ops.DveOp(name, spec, subdim=False, uops_sha=shas)
    dve_ops.OPS.append(op)
    dve_ops.CUSTOM_DVE_SPECS[name] = spec
    dve_ops._SUB_OPCODE_FOR_NAME[name] = opcode
    return op


RECIP1_MUL_GNN = _register_custom_op()



def build_nc(stage=99):
    nc = bacc.Bacc("TRN2", target_bir_lowering=False, debug=False)
    kv_ext = nc.declare_dram_parameter("kv", [BPC, N, CKV], F32, isOutput=False)
    pos_ext = nc.declare_dram_parameter("positions", [BPC, N, 3], F32, isOutput=False)
    out_ext = nc.declare_dram_parameter("out", [BPC, N, 3], F32, isOutput=True)

    with tile.TileContext(nc) as tc:
        with (
            tc.tile_pool(name="const", bufs=1) as constp,
            tc.tile_pool(name="kv16", bufs=2) as kv16p,
            tc.tile_pool(name="kvT", bufs=2) as kvTp,
            tc.tile_pool(name="aug", bufs=2) as augp,
            tc.tile_pool(name="augT", bufs=2) as augTp,
            tc.tile_pool(name="norm", bufs=4) as normp,
            tc.tile_pool(name="wt", bufs=16) as wtp,
            tc.tile_pool(name="epi", bufs=2) as epip,
            tc.tile_pool(name="psrel", bufs=2, space="PSUM") as psrel,
            tc.tile_pool(name="psd2", bufs=1, space="PSUM") as psd2,
            tc.tile_pool(name="psP", bufs=1, space="PSUM") as psP,
        ):
            # ---- PE warm-up primer: dependency-free back-to-back matmuls ----
            # (uninitialized operands on purpose: zero waits, so they issue at
            # t=0 and trip the HAM clock gate to 2.4 GHz during the DMA-bound
            # prologue; the product is never consumed mathematically)
            warm_in = constp.tile([128, 512], F16)
            nc.gpsimd.memset(warm_in[:, :], 0.0)
            warm_ps = psrel.tile([128, N], F32, tag="rel")
            for i in range(20):
                nc.tensor.matmul(
                    warm_ps[:, 0:512],
                    lhsT=warm_in[:, 0:128],
                    rhs=warm_in[:, :],
                    start=(i == 0),
                    stop=(i == 19),
                )
            warm_sink = constp.tile([128, 1], F32)
            nc.vector.tensor_copy(warm_sink[:, :], warm_ps[:, 0:1])

            identity16 = constp.tile([128, 128], F16)
            make_identity(nc, identity16[:, :])
            identity32 = constp.tile([128, 128], F32)
            make_identity(nc, identity32[:, :])
            dmask = constp.tile([128, 128], F16)
            nc.gpsimd.memset(dmask[:, :], 1.0)
            nc.gpsimd.affine_select(
                out=dmask[:, :],
                in_=dmask[:, :],
                compare_op=mybir.AluOpType.not_equal,
                fill=0.0,
                base=0,
                pattern=[[-1, 128]],
                channel_multiplier=1,
            )
            bias_tile = constp.tile([128, 1], F32)
            nc.gpsimd.memset(bias_tile[:, :], D2_BIAS)

            # pre-tanh values for both batches; one tanh at the end keeps a
            # single sqrt->tanh ACT-table transition for the whole kernel
            pre_all = constp.tile([128, BPC, NT, 3], F32)

            kmuT, vmuT, A16T, B16T, X, posf = {}, {}, {}, {}, {}, {}

            # ================= prologue: both batches =================
            for b in range(BPC):
                # ---- load kv, casting f32 -> fp16 in the SWDGE DMA ----
                kv16 = kv16p.tile([128, NT, 2 * E], F16, tag="kv16")
                nc.gpsimd.dma_start(
                    out=kv16[:, :, :],
                    in_=kv_ext[b].rearrange("(t p) c -> p t c", p=128)[:, :, 0 : 2 * E],
                )
                # ---- transpose k/v to [e, n] layout via PE ----
                kmuT[b] = kvTp.tile([128, N], F16, tag="kmuT", name=f"kmuT{b}")
                vmuT[b] = kvTp.tile([128, N], F16, tag="vmuT", name=f"vmuT{b}")
                kT_ps = psP.tile([128, N], F16, tag="P")
                for t in range(NT):
                    nc.tensor.transpose(
                        kT_ps[:, 128 * t : 128 * (t + 1)],
                        kv16[:, t, 0:E],
                        identity16[:, :],
                    )
                nc.vector.tensor_copy(kmuT[b][:, :], kT_ps[:, :])
                vT_ps = psP.tile([128, N], F16, tag="P")
                for t in range(NT):
                    nc.tensor.transpose(
                        vT_ps[:, 128 * t : 128 * (t + 1)],
                        kv16[:, t, E : 2 * E],
                        identity16[:, :],
                    )
                nc.vector.tensor_copy(vmuT[b][:, :], vT_ps[:, :])

                # ---- build augmented position blocks (n-major, fp16) ----
                # moving rows A: [a(3), b(3), a(3), b(3), 1,1,1, pn2 h/m/l]
                # stationary rows Bm: [-2a(3), -2a(3), -2b(3), -2b(3), pm2 h/m/l, 1,1,1]
                posf[b] = augp.tile([128, NT, 3], F32, tag="posf", name=f"posf{b}")
                nc.sync.dma_start(
                    out=posf[b][:, :, :],
                    in_=pos_ext[b].rearrange("(t p) d -> p t d", p=128),
                )
                pf = posf[b]
                A16 = augp.tile([128, NT, KA], F16, tag="A16")
                B16 = augp.tile([128, NT, KA], F16, tag="B16")
                sq3 = augp.tile([128, NT, 3], F32, tag="sq3")
                pn2 = augp.tile([128, NT, 1], F32, tag="pn2")
                t1 = augp.tile([128, NT, 1], F32, tag="t1")

                nc.vector.tensor_copy(A16[:, :, 0:3], pf[:, :, :])      # a
                nc.vector.tensor_sub(A16[:, :, 3:6], pf[:, :, :], A16[:, :, 0:3])
                nc.vector.tensor_copy(A16[:, :, 6:9], A16[:, :, 0:3])
                nc.vector.tensor_copy(A16[:, :, 9:12], A16[:, :, 3:6])
                nc.vector.memset(A16[:, :, 12:15], 1.0)
                nc.vector.tensor_mul(sq3[:, :, :], pf[:, :, :], pf[:, :, :])
                nc.vector.tensor_reduce(
                    out=pn2[:, :, :],
                    in_=sq3[:, :, :],
                    op=mybir.AluOpType.add,
                    axis=mybir.AxisListType.X,
                )
                nc.vector.tensor_copy(A16[:, :, 15:16], pn2[:, :, :])   # h
                nc.vector.tensor_sub(t1[:, :, :], pn2[:, :, :], A16[:, :, 15:16])
                nc.vector.tensor_copy(A16[:, :, 16:17], t1[:, :, :])    # m
                nc.vector.tensor_sub(t1[:, :, :], t1[:, :, :], A16[:, :, 16:17])
                nc.vector.tensor_copy(A16[:, :, 17:18], t1[:, :, :])    # l

                nc.vector.tensor_scalar_mul(B16[:, :, 0:3], A16[:, :, 0:3], -2.0)
                nc.vector.tensor_copy(B16[:, :, 3:6], B16[:, :, 0:3])
                nc.vector.tensor_scalar_mul(B16[:, :, 6:9], A16[:, :, 3:6], -2.0)
                nc.vector.tensor_copy(B16[:, :, 9:12], B16[:, :, 6:9])
                nc.vector.tensor_copy(B16[:, :, 12:15], A16[:, :, 15:18])
                nc.vector.memset(B16[:, :, 15:18], 1.0)

                X[b] = augp.tile([128, NT, 4], F16, tag="X", name=f"X{b}")
                nc.vector.tensor_copy(X[b][:, :, 0:3], A16[:, :, 0:3])
                nc.vector.memset(X[b][:, :, 3:4], 1.0)

                # ---- transpose aug blocks to [KA, N] via PE ----
                A_ps = psP.tile([KA, N], F16, tag="P")
                for t in range(NT):
                    nc.tensor.transpose(
                        A_ps[:, 128 * t : 128 * (t + 1)], A16[:, t, :], identity16[:, :]
                    )
                A16T[b] = augTp.tile([KA, N], F16, tag="A16T", name=f"A16T{b}")
                nc.vector.tensor_copy(A16T[b][:, :], A_ps[:, :])

                B_ps = psP.tile([KA, N], F16, tag="P")
                for t in range(NT):
                    nc.tensor.transpose(
                        B_ps[:, 128 * t : 128 * (t + 1)], B16[:, t, :], identity16[:, :]
                    )
                B16T[b] = augTp.tile([KA, N], F16, tag="B16T", name=f"B16T{b}")
                nc.scalar.copy(B16T[b][:, :], B_ps[:, :])

            # ============ main loop: interleave both batches ============
            # b0 leads so the loop never stalls on b1's prologue; b1 trails
            # and b0's deferred P-matmuls fill the PE while b1 finishes
            pair_order = [(0, 0), (0, 1), (0, 2)]
            rest0 = [(0, t) for t in range(3, NT)]
            rest1 = [(1, t) for t in range(NT)]
            while rest0 or rest1:
                if rest1:
                    pair_order.append(rest1.pop(0))
                if rest0:
                    pair_order.append(rest0.pop(0))
            wts = {b: [] for b in range(BPC)}
            for b, t in pair_order:
                    normt = normp.tile([128, N], F32)
                    wt = wtp.tile([128, N], F16)
                    d2_ps = psd2.tile([128, N], F32, tag="d2")
                    for h in range(2):
                        cs = slice(512 * h, 512 * (h + 1))
                        nc.tensor.matmul(
                            d2_ps[:, cs],
                            lhsT=B16T[b][:, 128 * t : 128 * (t + 1)],
                            rhs=A16T[b][:, cs],
                            start=True,
                            stop=True,
                        )
                    nc.scalar.activation(
                        normt[:, :],
                        d2_ps[:, :],
                        mybir.ActivationFunctionType.Sqrt,
                        bias=bias_tile[:, 0:1],
                        scale=1.0,
                    )
                    rel_ps = psrel.tile([128, N], F32, tag="rel")
                    for h in range(2):
                        cs = slice(512 * h, 512 * (h + 1))
                        nc.tensor.matmul(
                            rel_ps[:, cs],
                            lhsT=vmuT[b][:, 128 * t : 128 * (t + 1)],
                            rhs=kmuT[b][:, cs],
                            start=True,
                            stop=True,
                        )
                    nc.vector._custom_dve(
                        RECIP1_MUL_GNN,
                        out=wt[:, :],
                        in0=rel_ps[:, :],
                        in1=normt[:, :],
                        s0=C0_FOLD,
                        s1=C1_FOLD,
                    )
                    # zero the diagonal block (exact diag kill; also keeps the
                    # NaN->0 laundered entries harmless)
                    nc.gpsimd.tensor_mul(
                        wt[:, 128 * t : 128 * (t + 1)],
                        wt[:, 128 * t : 128 * (t + 1)],
                        dmask[:, :],
                    )
                    wts[b].append(wt)

            # ============ deferred P accumulation + epilogues ============
            for b in range(BPC):
                P_ps = psP.tile([4, N], F32, tag="P", name=f"P{b}")
                for t in range(NT):
                    for h in range(2):
                        cs = slice(512 * h, 512 * (h + 1))
                        nc.tensor.matmul(
                            P_ps[:, cs],
                            lhsT=X[b][:, t, :],
                            rhs=wts[b][t][:, cs],
                            start=(t == 0),
                            stop=(t == NT - 1),
                        )
                Psb = epip.tile([4, N], F32, tag="Psb")
                nc.scalar.copy(Psb[:, :], P_ps[:, :])
                PT_ps = psP.tile([128, NT * 4], F32, tag="P")
                for c in range(NT):
                    nc.tensor.transpose(
                        PT_ps[:, 4 * c : 4 * (c + 1)],
                        Psb[:, 128 * c : 128 * (c + 1)],
                        identity32[0:4, 0:4],
                    )
                PT = epip.tile([128, NT, 4], F32, tag="PT")
                nc.vector.tensor_copy(
                    PT[:, :, :], PT_ps[:, :].rearrange("p (t f) -> p t f", f=4)
                )
                tmp = epip.tile([128, NT, 3], F32, tag="tmp")
                a0, a1 = bass.broadcast_tensor_aps(posf[b][:, :, :], PT[:, :, 3:4])
                nc.gpsimd.tensor_mul(tmp[:, :, :], a0, a1)
                nc.gpsimd.tensor_sub(pre_all[:, b, :, :], tmp[:, :, :], PT[:, :, 0:3])

            # ---- single tanh + scale + store for both batches ----
            act = constp.tile([128, BPC, NT, 3], F32)
            nc.scalar.activation(
                act[:, :, :, :],
                pre_all[:, :, :, :],
                mybir.ActivationFunctionType.Tanh,
            )
            actf = constp.tile([128, BPC, NT, 3], F32)
            nc.gpsimd.tensor_scalar_mul(actf[:, :, :, :], act[:, :, :, :], ACTION_SCALE)
            for b in range(BPC):
                nc.sync.dma_start(
                    out=out_ext[b].rearrange("(t p) d -> p t d", p=128),
                    in_=actf[:, b, :, :],
                )

    nc.compile()
    return nc


_NC_CACHE = {}


def _get_nc():
    if "nc" not in _NC_CACHE:
        _NC_CACHE["nc"] = build_nc()
    return _NC_CACHE["nc"]


def kernel(**inputs):
    kv = np.ascontiguousarray(np.asarray(inputs["kv"], dtype=np.float32))
    pos = np.ascontiguousarray(np.asarray(inputs["positions"], dtype=np.float32))
    assert kv.shape == (B, N, CKV) and pos.shape == (B, N, 3)
    nc = _get_nc()
    in_maps = [
        {
            "kv": kv[i * BPC : (i + 1) * BPC],
            "positions": pos[i * BPC : (i + 1) * BPC],
        }
        for i in range(NCORES)
    ]
    res = run_bass_kernel_spmd(nc, in_maps, core_ids=list(range(NCORES)))
    outs = res.results
    return np.concatenate([outs[i]["out"] for i in range(NCORES)], axis=0)


if __name__ == "__main__":
    rng = np.random.default_rng(0)
    kv = rng.standard_normal((B, N, CKV), dtype=np.float32)
    pos = rng.standard_normal((B, N, 3), dtype=np.float32)
    out = kernel(kv=kv, positions=pos)
    print("out", out.shape, out.dtype, float(np.abs(out).max()))
